# revision 4
# baseline (speedup 1.0000x reference)
"""Trainium2 Bass kernel v3 for nn_GCNII_80178449482260 (2x dense GAT + GCNII).

Redesign vs v2 baseline:
  * GAT1 Wh column-sharded: core c computes x @ Wg1[h][:, c*128:(c+1)*128]
    (all rows, its column block) from a resident SBUF weight slice, instead
    of streaming the full 10.5MB Wg1 per core per rep.
  * attention a-vectors folded into the weights on the host:
    u = (x@W)@a1 == x@(W@a1) - u/v come from tiny 16-col matmuls (GAT1) or
    augmented weight columns (o1/g2/o2); no DVE uv_accum chains.
  * oT-form attention: out^T blocks accumulate via lhsT=Wh[k-chunk,c'-block],
    rhs=nT[:,k-block]; output lands pre-transposed as the next layer's lhsT
    (no per-attention PE transposes). Softmax denominator via a ones-lhsT
    matmul; 1/rowsum applied as a free-dim broadcast row.
  * h0 rides the g2 gather payload instead of being recomputed on all cores.
  * masks arrive pre-transposed/pre-scaled from the host (madjT input).
  * Wo2/fc1/Wg2/fc0/Wg1-slices/xT resident in SBUF; only Wo1 streams.

Self-contained: builds/compiles the Bass program on first call, caches it,
and runs via run_bass_kernel_spmd on cores 0-7.
"""
import os
import sys
import numpy as np

for _p in ("/opt/trn_rl_repo", "/root/.axon_site/_ro/trn_rl_repo"):
    if _p not in sys.path:
        sys.path.insert(0, _p)

import ml_dtypes  # noqa: E402
from concourse import bacc, tile, mybir  # noqa: E402
from concourse.bass_utils import run_bass_kernel_spmd  # noqa: E402
from concourse.kernels.tile_matmul import make_identity  # noqa: E402

BF16 = mybir.dt.bfloat16
F32 = mybir.dt.float32
AF = mybir.ActivationFunctionType
OP = mybir.AluOpType

N = 1024      # nodes
P = 128       # partitions / rows per core
C = 8         # cores
HID = 512
NC1 = 512
H1, H2 = 5, 2
TAIL = 16
PAYO1 = NC1 + TAIL + HID            # o1 payload (+h0)
PAYMAX = 2 * NC1 + TAIL             # g2 payload
WFCOLS = N + TAIL                   # pulled tile free size
THETA2 = float(np.log(1.25))        # GCNII layer-2 theta; layer 1 is dead code
SLOPE = 0.25
NEG = -9.0e15
RG = [list(range(C))]
_NO_CC = bool(int(os.environ.get("KERNEL_NO_CC", "0")))  # profiling stand-in
_DBG = bool(int(os.environ.get("KERNEL_DBG", "0")))

_CACHE = {}


def _build(reps=1):
    nc = bacc.Bacc("TRN2", target_bir_lowering=False, debug=False,
                   num_devices=C)
    d = {}
    d["xT"] = nc.dram_tensor("xT", [N, N], BF16, kind="ExternalInput")
    d["xT_sl"] = nc.dram_tensor("xT_sl", [N, P], BF16, kind="ExternalInput")
    d["x_nat"] = nc.dram_tensor("x_nat", [N, N], BF16, kind="ExternalInput")
    d["Wg1"] = nc.dram_tensor("Wg1", [H1, N, N], BF16, kind="ExternalInput")
    d["qg2"] = nc.dram_tensor("qg2", [4, NC1], BF16, kind="ExternalInput")
    d["pvec"] = nc.dram_tensor("pvec", [N, TAIL], BF16, kind="ExternalInput")
    d["madjT"] = nc.dram_tensor("madjT", [N, P], BF16, kind="ExternalInput")
    d["wo1_aug"] = nc.dram_tensor("wo1_aug", [H1 * N, NC1 + TAIL], BF16,
                                  kind="ExternalInput")
    d["wg2_aug"] = nc.dram_tensor("wg2_aug", [H2, NC1, NC1 + TAIL], BF16,
                                  kind="ExternalInput")
    d["wo2_aug"] = nc.dram_tensor("wo2_aug", [N, N + TAIL], BF16,
                                  kind="ExternalInput")
    d["fc0_w"] = nc.dram_tensor("fc0_w", [N, HID], BF16, kind="ExternalInput")
    d["fc0_b"] = nc.dram_tensor("fc0_b", [HID], BF16, kind="ExternalInput")
    d["fc1_w"] = nc.dram_tensor("fc1_w", [HID, N], BF16, kind="ExternalInput")
    d["fc1_b"] = nc.dram_tensor("fc1_b", [N], BF16, kind="ExternalInput")
    d["cw1T_sl"] = nc.dram_tensor("cw1T_sl", [N, P], BF16, kind="ExternalInput")
    out_d = nc.dram_tensor("out", [P, N], F32, kind="ExternalOutput")
    if _DBG:
        d["dbg"] = nc.dram_tensor("dbg", [P, 10240], BF16, kind="ExternalOutput")

    with tile.TileContext(nc) as tc:
        _body(nc, tc, d, out_d, reps)
    nc.compile()
    return nc


def _body(nc, tc, d, out_d, reps=1):
    with (
        tc.tile_pool(name="cst", bufs=1) as cst,
        tc.tile_pool(name="wres", bufs=1) as wres,      # resident weights
        tc.tile_pool(name="per", bufs=1) as per,        # cross-phase persistents
        tc.tile_pool(name="wfp", bufs=3) as wf_p,       # pulled gather tiles
        tc.tile_pool(name="wch", bufs=4) as wch_p,      # weight chunk stream
        tc.tile_pool(name="nbfp", bufs=2) as nbf_p,     # attention nT tiles
        tc.tile_pool(name="mtp", bufs=2) as mt_p,       # A@x intermediates
        tc.tile_pool(name="payp", bufs=2) as pay_p,     # gather payload packs
        tc.tile_pool(name="sc32", bufs=2) as sc_32,     # f32 scratch
        tc.tile_pool(name="scbf", bufs=2) as sc_bf,     # bf16 scratch
        tc.tile_pool(name="bcp", bufs=3) as bc_p,       # broadcast rows (u/r)
        tc.tile_pool(name="smt", bufs=2) as sm,         # tiny per-row vecs
        tc.tile_pool(name="pswh", bufs=2, space="PSUM") as ps_wh,   # 4 banks
        tc.tile_pool(name="psat", bufs=1, space="PSUM") as ps_at,   # 2 banks
        tc.tile_pool(name="psac", bufs=1, space="PSUM") as ps_ac,   # 1 bank
        tc.tile_pool(name="pssm", bufs=1, space="PSUM") as ps_sm,   # 1 bank
        tc.tile_pool(name="dram", bufs=1, space="DRAM") as dram,
    ):
        ident = cst.tile([P, P], BF16, tag="ident")
        make_identity(nc, ident)
        ident32 = cst.tile([P, P], F32, tag="ident32")
        make_identity(nc, ident32)
        ones = cst.tile([P, 1], BF16, tag="ones")
        nc.vector.memset(ones[:], 1.0)

        # ---------------- resident inputs ----------------
        xtf = wres.tile([P, C, N], BF16, tag="xtf")      # xT[c*128+p, m]
        nc.sync.dma_start(xtf[:], d["xT"].ap().rearrange("(c p) m -> p c m", p=P))
        xtsl = wres.tile([P, C, P], BF16, tag="xtsl")    # xT[:, my rows]
        nc.sync.dma_start(xtsl[:], d["xT_sl"].ap().rearrange("(c p) m -> p c m", p=P))
        xnat = wres.tile([P, C, N], BF16, tag="xnat")    # x[k*128+p, f]
        nc.sync.dma_start(xnat[:], d["x_nat"].ap().rearrange("(k p) f -> p k f", p=P))
        qo2sb = wres.tile([P, C, 2], BF16, tag="qo2sb")  # Wo2@ao2 halves
        nc.sync.dma_start(qo2sb[:],
                          d["wo2_aug"].ap().rearrange("(c p) f -> p c f", p=P)[:, :, N:N + 2])
        qo1sb = wres.tile([P, H1 * C, 2], BF16, tag="qo1sb")  # Wo1@ao1 halves
        nc.sync.dma_start(qo1sb[:],
                          d["wo1_aug"].ap().rearrange("(g p) f -> p g f", p=P)[:, :, NC1:NC1 + 2])
        pv = wres.tile([P, C, TAIL], BF16, tag="pv")
        nc.sync.dma_start(pv[:], d["pvec"].ap().rearrange("(c p) t -> p c t", p=P))
        madjT = wres.tile([P, C, P], BF16, tag="madjT")
        nc.sync.dma_start(madjT[:], d["madjT"].ap().rearrange("(c p) i -> p c i", p=P))
        wg2 = wres.tile([P, H2, 4, NC1 + TAIL], BF16, tag="wg2")
        for h in range(H2):
            nc.sync.dma_start(wg2[:, h],
                              d["wg2_aug"].ap()[h].rearrange("(c p) f -> p c f", p=P))
        fc0 = wres.tile([P, C, HID], BF16, tag="fc0")
        nc.sync.dma_start(fc0[:], d["fc0_w"].ap().rearrange("(c p) f -> p c f", p=P))
        cw1 = wres.tile([P, C, P], BF16, tag="cw1")
        nc.sync.dma_start(cw1[:], d["cw1T_sl"].ap().rearrange("(c p) m -> p c m", p=P))

        def prep_bias(src_ap, L, tag):
            b_sb = sm.tile([1, N], BF16, tag="brow", bufs=1)
            nc.gpsimd.dma_start(b_sb[:1, :L], src_ap[None, :])
            b_bc = wres.tile([P, L], BF16, tag=tag)
            nc.gpsimd.partition_broadcast(b_bc[:], b_sb[:1, :L])
            return b_bc

        b_fc0 = prep_bias(d["fc0_b"].ap(), HID, "b_fc0")
        b_fc1 = prep_bias(d["fc1_b"].ap(), N, "b_fc1")
        q_g2 = [prep_bias(d["qg2"].ap()[t], NC1, f"qg2_{t}") for t in range(4)]

        # ---------------- persistents ----------------
        hcatT = per.tile([P, H1 * C, P], BF16, tag="hcatT")   # GAT1 heads outT
        xgrow = per.tile([P, NC1], BF16, tag="xgrow")         # o1 out rows
        hcat2T = per.tile([P, C, P], BF16, tag="hcat2T")      # GAT2 heads outT
        xg2T = per.tile([P, C, P], BF16, tag="xg2T")          # o2 outT
        h0f = per.tile([P, HID], F32, tag="h0f")              # my rows, f32
        h0full = per.tile([P, C, HID], BF16, tag="h0full")    # all rows (pulled)
        hT = per.tile([P, 4, P], BF16, tag="hT")
        val = per.tile([P, C, TAIL], F32, tag="val")          # v_all (g1 heads)
        sf = per.tile([P, HID], F32, tag="sf")

        def gather(pay_ap, cols, tag):
            """pay_ap: SBUF [P, cols] bf16 -> allgather -> shared [C*P, cols]."""
            ag_in = dram.tile([P, cols], BF16, tag=f"agi_{tag}")
            if _NO_CC:
                ag_out = dram.tile([C * P, cols], BF16, tag=f"ago_{tag}")
            else:
                ag_out = dram.tile([C * P, cols], BF16, tag=f"ago_{tag}",
                                   addr_space="Shared")
            for c0 in range(0, cols, 544):
                w = min(544, cols - c0)
                nc.gpsimd.dma_start(ag_in[:, c0:c0 + w], pay_ap[:, c0:c0 + w])
            if _NO_CC:
                for cc in range(C):
                    nc.gpsimd.dma_start(ag_out[cc * P:(cc + 1) * P, :], ag_in[:])
            else:
                nc.gpsimd.collective_compute(
                    "AllGather", OP.bypass, replica_groups=RG,
                    ins=[ag_in.opt()], outs=[ag_out.opt()])
            return ag_out

        def copy_ps(dst, src, idx=0):
            if idx % 2 == 0:
                nc.scalar.activation(dst, src, AF.Copy)
            else:
                nc.vector.tensor_copy(dst, src)

        def bcast_row(src_row):
            b = bc_p.tile([P, P], BF16, tag="bcrow")
            nc.gpsimd.partition_broadcast(b[:], src_row)
            return b

        def bcast_row32(src_row):
            b = bc_p.tile([P, P], F32, tag="bcrow32")
            nc.gpsimd.partition_broadcast(b[:], src_row)
            return b

        def attention(wf_lhsT, nblk, u_bc, v_col_fn, out_T, out_off, smp,
                      tagid=""):
            """oT-form attention.

            wf_lhsT(cb, k) -> [128,128] lhsT AP = Wh[k*128:(k+1)*128,
                cb*128:(cb+1)*128] (rows=nodes of chunk k on partitions).
            nblk = F // 128 output blocks. u_bc: [P, P] bf16 (u along free).
            v_col_fn(k) -> [P, 1] AP with v for nodes k*128+p.
            Writes scaled+elu'd oT into out_T[:, out_off:out_off+nblk, :].
            """
            eT = sc_32.tile([P, N], F32, tag="s32")
            for k in range(C):
                nc.vector.scalar_tensor_tensor(
                    eT[:, k * P:(k + 1) * P], u_bc[:, :P], v_col_fn(k),
                    madjT[:, k, :], op0=OP.add, op1=OP.add)
            nT = nbf_p.tile([P, N], BF16, tag="nbf")
            for s0 in range(0, N, 512):
                nc.vector.scalar_tensor_tensor(eT[:, s0:s0 + 512], eT[:, s0:s0 + 512],
                                               0.01, eT[:, s0:s0 + 512],
                                               op0=OP.mult, op1=OP.max)
                nc.scalar.activation(nT[:, s0:s0 + 512], eT[:, s0:s0 + 512], AF.Exp)
            # rowsum via ones-lhsT matmul -> [1, P] free-dim layout
            rs_ps = smp[:, 0:P]
            for k in range(C):
                nc.tensor.matmul(rs_ps[0:1], ones[:], nT[:, k * P:(k + 1) * P],
                                 start=(k == 0), stop=(k == C - 1))
            rrow = sm.tile([1, P], F32, tag="rrow", bufs=2)
            nc.vector.reciprocal(rrow[:1, :], rs_ps[0:1])
            rT_bc = bcast_row32(rrow[:1, :])
            # att matmuls: oT block cb accumulates over node chunks k
            oT_ps = ps_at.tile([P, C, P], F32, tag="otps")
            for cb in range(nblk):
                for k in range(C):
                    nc.tensor.matmul(oT_ps[:, cb, :], wf_lhsT(cb, k),
                                     nT[:, k * P:(k + 1) * P],
                                     start=(k == 0), stop=(k == C - 1))
            # scale by 1/rowsum (free-dim broadcast), elu, write bf16 oT
            t32 = sc_32.tile([P, N], F32, tag="s32b")
            for cb in range(nblk):
                nc.vector.scalar_tensor_tensor(t32[:, cb * P:(cb + 1) * P],
                                               oT_ps[:, cb, :], 1.0, rT_bc[:, :P],
                                               op0=OP.mult, op1=OP.mult)
            F = nblk * P
            m_bf = sc_bf.tile([P, N], BF16, tag="elum")
            nc.vector.tensor_scalar(m_bf[:, :F], t32[:, :F], 0.0, None, op0=OP.min)
            g32 = sc_32.tile([P, N], F32, tag="s32", name="elug")
            nc.scalar.activation(g32[:, :F], m_bf[:, :F], AF.Exp)
            nc.scalar.activation(m_bf[:, :F], t32[:, :F], AF.Relu)
            ofl = out_T[:, out_off:out_off + nblk, :].rearrange("p b q -> p (b q)")
            nc.vector.scalar_tensor_tensor(ofl, g32[:, :F], -1.0,
                                           m_bf[:, :F], op0=OP.add, op1=OP.add)

        def elu32(ps_flat, F, o32, nm=""):
            m_bf = sc_bf.tile([P, N], BF16, tag="elum", name=f"em{nm}")
            nc.vector.tensor_scalar(m_bf[:, :F], ps_flat[:, :F], 0.0, None,
                                    op0=OP.min)
            g32 = sc_32.tile([P, N], F32, tag="s32", name=f"eg{nm}")
            nc.scalar.activation(g32[:, :F], m_bf[:, :F], AF.Exp)
            nc.scalar.activation(m_bf[:, :F], ps_flat[:, :F], AF.Relu)
            nc.vector.scalar_tensor_tensor(o32[:, :F], g32[:, :F], -1.0,
                                           m_bf[:, :F], op0=OP.add, op1=OP.add)

        def trsp_store(o32, nblk, out_T, out_off, smp):
            for b in range(nblk):
                tp = smp[:, 384:512]
                nc.tensor.transpose(tp, o32[:, b * P:(b + 1) * P], ident32[:])
                copy_ps(out_T[:, out_off + b, :], tp, b)

        def mm2_oT(oT2, wcs, mT, nblk):
            for fb in range(nblk):
                for G in range(nblk):
                    nc.tensor.matmul(oT2[:, fb, :],
                                     wcs[G][:, fb * P:(fb + 1) * P],
                                     mT[:, G, :],
                                     start=(G == 0), stop=(G == nblk - 1))

        def u_broadcast(uv_col_f32, smp):
            """[P, 1] f32 column -> [P, P] bf16 broadcast along partitions."""
            tp = smp[:, 384:512]
            nc.tensor.transpose(tp[0:1], uv_col_f32, ident32[:])
            ur = sm.tile([1, P], BF16, tag="ubrow", bufs=2)
            nc.vector.tensor_copy(ur[:1, :], tp[0:1])
            return bcast_row(ur[:1, :])

        for _rep in range(reps):
            smp = ps_sm.tile([P, 512], F32, tag="sm")
            # ======== phase W: GAT1 col-sharded Whs + gathers ========
            g1_ag = []
            for h in range(H1):
                whp = ps_wh.tile([P, C, P], F32, tag="whps")
                for k in range(C):
                    for c2 in range(C):
                        nc.tensor.matmul(whp[:, k, :], xtf[:, c2, k * P:(k + 1) * P],
                                         wgsl[:, h, c2, :],
                                         start=(c2 == 0), stop=(c2 == C - 1))
                pay = pay_p.tile([P, PAYMAX], BF16, tag="pay")
                whpf = whp[:].rearrange("p k q -> p (k q)")
                copy_ps(pay[:, :512], whpf[:, :512], h)
                copy_ps(pay[:, 512:1024], whpf[:, 512:1024], h + 1)
                g1_ag.append(gather(pay[:, :C * P], C * P, f"g1_{h}"))

            # my u/v for all g1 heads ([P,16]: col 2h=u_h, 2h+1=v_h)
            uvp = smp[:, 128:128 + TAIL]
            for c2 in range(C):
                nc.tensor.matmul(uvp, xtsl[:, c2, :], pv[:, c2, :],
                                 start=(c2 == 0), stop=(c2 == C - 1))
            uvf = sm.tile([P, TAIL], F32, tag="uvf", bufs=2)
            nc.vector.tensor_copy(uvf[:], uvp)

            # v_all for g1 heads: val[p, k, 2h+1] = v_h(node k*128+p)
            for k in range(C):
                for c2 in range(C):
                    nc.tensor.matmul(smp[:, 256 + TAIL * k:256 + TAIL * (k + 1)],
                                     xtf[:, c2, k * P:(k + 1) * P],
                                     pv[:, c2, :],
                                     start=(c2 == 0), stop=(c2 == C - 1))
            nc.vector.tensor_copy(val[:].rearrange("p k t -> p (k t)"),
                                  smp[:, 256:256 + C * TAIL])

            # my-rows h0 (f32, exact) - feeds the GCNII combines
            h0p = ps_wh.tile([P, C, P], F32, tag="whps", name="h0p")
            h0pf = h0p[:].rearrange("p k q -> p (k q)")
            for c2 in range(C):
                nc.tensor.matmul(h0pf[:, :HID], xtsl[:, c2, :], fc0[:, c2, :],
                                 start=(c2 == 0), stop=(c2 == C - 1))
            nc.vector.scalar_tensor_tensor(h0f[:], h0pf[:, :HID], 1.0,
                                           b_fc0[:, :HID], op0=OP.mult, op1=OP.add)
            nc.vector.scalar_tensor_tensor(h0f[:], h0f[:], SLOPE, h0f[:],
                                           op0=OP.mult, op1=OP.max)
            h0b = sc_bf.tile([P, HID], BF16, tag="h0b")
            copy_ps(h0b[:], h0f[:], 1)

            # ======== phase A: GAT1 attentions + interleaved o1 Wh ========
            o1ps = ps_ac.tile([P, NC1], F32, tag="acc", name="o1ps")
            o1uv = smp[:, 144:146]

            for h in range(H1):
                wf = wf_p.tile([P, C, WFCOLS], BF16, tag="wfull")
                ag3 = g1_ag[h][:].rearrange("(c p) f -> p c f", p=P)
                for c in range(C):
                    nc.sync.dma_start(wf[:, c, :N], ag3[:, c, :])
                u_bc = u_broadcast(uvf[:, 2 * h:2 * h + 1], smp)
                attention(lambda cb, k, _wf=wf: _wf[:, cb, k * P:(k + 1) * P], C,
                          u_bc,
                          lambda k, _h=h: val[:, k, 2 * _h + 1:2 * _h + 2],
                          hcatT, h * C, smp, tagid=f"g1{h}")
                for t in range(C):
                    g = h * C + t
                    wch = wch_p.tile([P, N + TAIL], BF16, tag="wch",
                                     name=f"wo1c{g}")
                    nc.sync.dma_start(
                        wch[:, :NC1 + TAIL], d["wo1_aug"].ap()
                        .rearrange("(g p) f -> p g f", p=P)[:, g, :])
                    nc.tensor.matmul(o1ps[:], hcatT[:, g, :], wch[:, :NC1],
                                     start=(g == 0), stop=(g == H1 * C - 1))
                    nc.tensor.matmul(o1uv, hcatT[:, g, :], wch[:, NC1:NC1 + TAIL],
                                     start=(g == 0), stop=(g == H1 * C - 1))

            # ======== o1: pack + gather + attention ========
            pay = pay_p.tile([P, PAYMAX], BF16, tag="pay", name="payo1")
            nc.scalar.activation(pay[:, :NC1], o1ps[:], AF.Copy)
            nc.vector.tensor_copy(pay[:, NC1:NC1 + 2], o1uv)
            copy_ps(pay[:, NC1 + TAIL:PAYO1], h0b[:], 1)
            ag_o1 = gather(pay[:, :PAYO1], PAYO1, "o1")
            uvo1 = sm.tile([P, TAIL], F32, tag="uvf", bufs=2, name="uvo1")
            nc.vector.tensor_copy(uvo1[:, 0:2], o1uv)

            wf2 = wf_p.tile([P, C, WFCOLS], BF16, tag="wfull", name="wfo1")
            ag3 = ag_o1[:].rearrange("(c p) f -> p c f", p=P)
            for j in range(C):
                nc.sync.dma_start(wf2[:, j, :NC1 + TAIL],
                                  ag3[:, j, :NC1 + TAIL])
                nc.sync.dma_start(h0full[:, j:j + 1, :],
                                  ag3[:, j:j + 1, NC1 + TAIL:])
            u_bc = u_broadcast(uvo1[:, 0:1], smp)
            attention(lambda cb, k, _w=wf2: _w[:, k, cb * P:(cb + 1) * P], 4, u_bc,
                      lambda k, _w=wf2: _w[:, k, NC1 + 1:NC1 + 2],
                      xgT, 0, smp, tagid="o1")

            # ======== GAT2 heads: joint Wh + one combined gather (+h0) ========
            pay2 = pay_p.tile([P, PAYMAX], BF16, tag="pay", name="payg2")
            g2uv = sm.tile([P, TAIL], F32, tag="uvf", bufs=2, name="g2uv")
            for h in range(H2):
                g2p = ps_wh.tile([P, C, P], F32, tag="whps", name=f"g2p{h}")
                g2pf = g2p[:].rearrange("p k q -> p (k q)")
                g2u = smp[:, 160 + TAIL * h:160 + TAIL * (h + 1)]
                for c2 in range(4):
                    nc.tensor.matmul(g2pf[:, :NC1], xgT[:, c2, :],
                                     wg2[:, h, c2, :NC1],
                                     start=(c2 == 0), stop=(c2 == 3))
                    nc.tensor.matmul(g2u, xgT[:, c2, :], wg2[:, h, c2, NC1:],
                                     start=(c2 == 0), stop=(c2 == 3))
                copy_ps(pay2[:, h * NC1:(h + 1) * NC1], g2pf[:, :NC1], h)
                nc.vector.tensor_copy(pay2[:, 2 * NC1 + 2 * h:2 * NC1 + 2 * h + 2],
                                      g2u[:, 0:2])
                nc.vector.tensor_copy(g2uv[:, 2 * h:2 * h + 2], g2u[:, 0:2])
            ag_g2 = gather(pay2[:, :PAYMAX], PAYMAX, "g2")

            wf2g = wf_p.tile([P, C, WFCOLS], BF16, tag="wfull", name="wfg2")
            ag3 = ag_g2[:].rearrange("(c p) f -> p c f", p=P)
            for j in range(C):
                nc.sync.dma_start(wf2g[:, j, :2 * NC1 + TAIL],
                                  ag3[:, j, :2 * NC1 + TAIL])

            # o2 Wh accumulates per-head right after each g2 attention
            o2ps = ps_wh.tile([P, C, P], F32, tag="whps", name="o2ps")
            o2pf = o2ps[:].rearrange("p k q -> p (k q)")
            o2uv = smp[:, 192:194]
            for h in range(H2):
                u_bc = u_broadcast(g2uv[:, 2 * h:2 * h + 1], smp)
                attention(
                    lambda cb, k, _w=wf2g, _h=h: _w[:, k, _h * NC1 + cb * P:
                                                    _h * NC1 + (cb + 1) * P],
                    4, u_bc,
                    lambda k, _w=wf2g, _h=h: _w[:, k, 2 * NC1 + 2 * _h + 1:
                                                2 * NC1 + 2 * _h + 2],
                    hcat2T, h * 4, smp, tagid=f"g2{h}")
                for t in range(4):
                    g = h * 4 + t
                    wch = wch_p.tile([P, N + TAIL], BF16, tag="wch")
                    nc.sync.dma_start(
                        wch[:], d["wo2_aug"].ap()
                        .rearrange("(g p) f -> p g f", p=P)[:, g, :])
                    for s in (0, 512):
                        nc.tensor.matmul(o2pf[:, s:s + 512], hcat2T[:, g, :],
                                         wch[:, s:s + 512],
                                         start=(g == 0), stop=(g == C - 1))
                    nc.tensor.matmul(o2uv, hcat2T[:, g, :], wch[:, N:],
                                     start=(g == 0), stop=(g == C - 1))

            # ======== o2: pack + gather + attention ========
            pay = pay_p.tile([P, PAYMAX], BF16, tag="pay", name="payo2")
            copy_ps(pay[:, :512], o2pf[:, :512], 0)
            copy_ps(pay[:, 512:1024], o2pf[:, 512:1024], 1)
            nc.vector.tensor_copy(pay[:, N:N + TAIL], o2uv)
            ag_o2 = gather(pay[:, :N + TAIL], N + TAIL, "o2")
            uvo2 = sm.tile([P, TAIL], F32, tag="uvf", bufs=2, name="uvo2")
            nc.vector.tensor_copy(uvo2[:, 0:2], o2uv)

            wf2 = wf_p.tile([P, C, WFCOLS], BF16, tag="wfull", name="wfo2")
            ag3 = ag_o2[:].rearrange("(c p) f -> p c f", p=P)
            for j in range(C):
                nc.sync.dma_start(wf2[:, j, :N + TAIL], ag3[:, j, :])
            u_bc = u_broadcast(uvo2[:, 0:1], smp)
            attention(lambda cb, k, _w=wf2: _w[:, k, cb * P:(cb + 1) * P], C, u_bc,
                      lambda k, _w=wf2: _w[:, k, N + 1:N + 2],
                      xg2T, 0, smp, tagid="o2")

            # hi = xg2 @ h0_full
            hip = ps_ac.tile([P, NC1], F32, tag="acc", name="hip")
            for t in range(C):
                nc.tensor.matmul(hip[:], xg2T[:, t, :], h0full[:, t, :],
                                 start=(t == 0), stop=(t == C - 1))

            # ======== GCNII tail ========
            # sf kept UNSCALED (0.9*hi+0.1*h0 absorbed into downstream scalars)
            nc.vector.scalar_tensor_tensor(sf[:], hip[:], 9.0, h0f[:],
                                           op0=OP.mult, op1=OP.add)
            sb_bf = sc_bf.tile([P, HID], BF16, tag="h0b", name="sbbf")
            nc.vector.tensor_scalar(sb_bf[:], sf[:], 0.1, None, op0=OP.mult)
            ag_s = gather(sb_bf[:, :HID], HID, "s")
            s_fl = wf_p.tile([P, C, WFCOLS], BF16, tag="wfull", name="wfs")
            ag3s = ag_s[:].rearrange("(c p) f -> p c f", p=P)
            mmp = ps_ac.tile([P, NC1], F32, tag="acc", name="mmp")
            for j0 in range(0, C, 2):
                nc.sync.dma_start(s_fl[:, j0:j0 + 2, :HID], ag3s[:, j0:j0 + 2, :])
                for c in (j0, j0 + 1):
                    nc.tensor.matmul(mmp[:], cw1[:, c, :], s_fl[:, c, :HID],
                                     start=(c == 0), stop=(c == C - 1))
            # h = lrelu(theta*mm + (1-theta)*0.1*sf_raw + h0)
            hf = sc_32.tile([P, N], F32, tag="s32", name="hf")
            nc.vector.scalar_tensor_tensor(hf[:, :HID], sf[:],
                                           0.1 * (1.0 - THETA2) / THETA2,
                                           mmp[:], op0=OP.mult, op1=OP.add)
            nc.vector.scalar_tensor_tensor(hf[:, :HID], hf[:, :HID], THETA2, h0f[:],
                                           op0=OP.mult, op1=OP.add)
            hlr = sc_32.tile([P, N], F32, tag="s32b", name="hlr")
            nc.vector.scalar_tensor_tensor(hlr[:, :HID], hf[:, :HID], SLOPE,
                                           hf[:, :HID], op0=OP.mult, op1=OP.max)

            # fc1 via 4 transposed chunks
            for t in range(4):
                tp = smp[:, 384:512]
                nc.tensor.transpose(tp, hlr[:, t * P:(t + 1) * P], ident32[:])
                copy_ps(hT[:, t, :], tp, t)
            fc1_c = []
            for c in range(4):
                t = wch_p.tile([P, N + TAIL], BF16, tag="wch", name=f"fc1c{c}")
                nc.sync.dma_start(
                    t[:, :N],
                    d["fc1_w"].ap().rearrange("(c p) f -> p c f", p=P)[:, c, :])
                fc1_c.append(t)
            y_sb = sc_32.tile([P, N], F32, tag="s32b", name="ysb")
            for s in (0, 512):
                yp = ps_ac.tile([P, NC1], F32, tag="acc", name=f"yp{s}")
                for c in range(4):
                    nc.tensor.matmul(yp[:], hT[:, c, :], fc1_c[c][:, s:s + 512],
                                     start=(c == 0), stop=(c == 3))
                nc.vector.scalar_tensor_tensor(y_sb[:, s:s + 512], yp[:], 1.0,
                                               b_fc1[:, s:s + 512],
                                               op0=OP.mult, op1=OP.add)
            nc.sync.dma_start(out_d.ap(), y_sb[:])
            if _DBG:
                dfl = d["dbg"].ap()
                nc.sync.dma_start(dfl[:, 0:5120],
                                  hcatT[:].rearrange("p b q -> p (b q)"))
                nc.sync.dma_start(dfl[:, 5120:5632], xgrow[:])
                nc.sync.dma_start(dfl[:, 5632:6656],
                                  hcat2T[:].rearrange("p b q -> p (b q)"))
                nc.sync.dma_start(dfl[:, 6656:7680],
                                  xg2T[:].rearrange("p b q -> p (b q)"))
                nc.gpsimd.dma_start(dfl[:, 7680:7696], uvf[:])
                nc.gpsimd.dma_start(dfl[:, 7696:7824],
                                    val[:].rearrange("p k t -> p (k t)"))


def _shard_inputs(inputs):
    f32 = lambda a: np.ascontiguousarray(np.asarray(a, dtype=np.float32))
    bf = lambda a: np.ascontiguousarray(np.asarray(a, dtype=np.float32)).astype(
        ml_dtypes.bfloat16)
    x = f32(inputs["x"])
    adj = f32(inputs["adj"])
    xT_bf = np.ascontiguousarray(bf(x).T)
    Wg1 = f32(inputs["Wg1"])
    ag1 = f32(inputs["ag1"])[:, :, 0]        # [5, 2048]
    Wo1 = f32(inputs["Wo1"])
    ao1 = f32(inputs["ao1"])[:, 0]           # [1024]
    Wg2 = f32(inputs["Wg2"])
    ag2 = f32(inputs["ag2"])[:, :, 0]        # [2, 1024]
    Wo2 = f32(inputs["Wo2"])
    ao2 = f32(inputs["ao2"])[:, 0]           # [2048]
    cw1T = np.ascontiguousarray(bf(inputs["cw1"]).T)

    pvec = np.zeros((N, TAIL), np.float32)
    for h in range(H1):
        pvec[:, 2 * h] = Wg1[h] @ ag1[h][:N]
        pvec[:, 2 * h + 1] = Wg1[h] @ ag1[h][N:]

    wo1_aug = np.zeros((H1 * N, NC1 + TAIL), np.float32)
    wo1_aug[:, :NC1] = Wo1
    wo1_aug[:, NC1] = Wo1 @ ao1[:NC1]        # u
    wo1_aug[:, NC1 + 1] = Wo1 @ ao1[NC1:]    # v

    wg2_aug = np.zeros((H2, NC1, NC1 + TAIL), np.float32)
    wg2_aug[:, :, :NC1] = Wg2
    for h in range(H2):
        wg2_aug[h, :, NC1] = Wg2[h] @ ag2[h][:NC1]
        wg2_aug[h, :, NC1 + 1] = Wg2[h] @ ag2[h][NC1:]

    wo2_aug = np.zeros((N, N + TAIL), np.float32)
    wo2_aug[:, :N] = Wo2
    wo2_aug[:, N] = Wo2 @ ao2[:N]
    wo2_aug[:, N + 1] = Wo2 @ ao2[N:]

    qg2 = np.zeros((4, NC1), np.float32)
    for h in range(H2):
        qg2[2 * h] = Wg2[h] @ ag2[h][:NC1]
        qg2[2 * h + 1] = Wg2[h] @ ag2[h][NC1:]

    shared = {
        "xT": xT_bf,
        "x_nat": bf(x),
        "Wg1": bf(Wg1),
        "qg2": bf(qg2),
        "pvec": bf(pvec),
        "wo1_aug": bf(wo1_aug),
        "wg2_aug": bf(wg2_aug),
        "wo2_aug": bf(wo2_aug),
        "fc0_w": bf(inputs["fc0_w"]),
        "fc0_b": bf(inputs["fc0_b"]),
        "fc1_w": bf(inputs["fc1_w"]),
        "fc1_b": bf(inputs["fc1_b"]),
    }
    in_maps = []
    for c in range(C):
        r0, r1 = c * P, (c + 1) * P
        m = dict(shared)
        m["xT_sl"] = np.ascontiguousarray(xT_bf[:, r0:r1])
        # madjT[n, i] = 0 if adj[my_i, n] > 0 else NEG
        madjT = np.where(adj[r0:r1].T > 0, 0.0, NEG).astype(np.float32)
        m["madjT"] = bf(madjT)
        m["cw1T_sl"] = np.ascontiguousarray(cw1T[:, r0:r1])
        in_maps.append(m)
    return in_maps


def kernel(**inputs) -> np.ndarray:
    if "nc" not in _CACHE:
        _CACHE["nc"] = _build()
    nc = _CACHE["nc"]
    in_maps = _shard_inputs(inputs)
    res = run_bass_kernel_spmd(nc, in_maps, core_ids=list(range(C)))
    out = np.concatenate([res.results[c]["out"] for c in range(C)], axis=0)
    if _DBG:
        kernel.dbg = [np.asarray(res.results[c]["dbg"], dtype=np.float32)
                      for c in range(C)]
    return np.asarray(out, dtype=np.float32)
